# revision 19
# baseline (speedup 1.0000x reference)
"""BitLinear (activation int8-quant + ternary weight) Trainium2 kernel.

Strategy (8 NeuronCores, token-parallel):
  - x [2,8192,2048] -> flat [16384, 2048]; core c gets a contiguous slice of
    2048 tokens (natural [token, feature] layout).
  - weight is TERNARIZED ON HOST exactly as the reference does (jax-CPU
    w_scale = mean|W|, strict f32 compares against +-0.5*w_scale), then
    shipped host-transposed as fp8e4 wqt = w_q.T in {-1,0,+1} ([in, out],
    4MB instead of 16MB f32).  This is the standard BitNet deployment
    contract (ternary weights are a precomputed artifact of the layer).
    The PE consumes the fp8 tiles directly as the moving matmul operand
    against the bf16 x_q stationary operand (verified exact: all values are
    small integers).
  - Activation quantization stays on device and is bit-exact to the
    reference: per-token absmax (DVE), exact int8 round via the magic-number
    trick (ACT magic-add + DVE magic-sub/bf16-cast), per-token scales
    replicated into the output rescale gf = w_scale/s.
  - bf16/fp8 PE matmuls with exact small-integer operands, fp32 PSUM
    accumulation => bit-exact integer GEMM.  out = square(relu(gf*acc)):
    Relu (scale=gf) on ACT, Square on DVE, one batched [128,2048] output
    DMA per token block (per 512-chunk for the last block to cut the tail).

v4 schedule notes (what the v3 trace taught):
  - The kernel is PE-bound: 1024 N=512 matmuls ~= 221us at 2.4GHz; every
    other engine is sized to stay off the PE's critical path.
  - Tile multiplexes ALL DMAs over 8 shared HW completion semaphores and
    every ucode DMA-transpose acts as a *bidirectional barrier* against
    other DMAs in scheduled order.  Therefore every DMA (loads, transposes,
    stores) is emitted on the ONE sync queue in an explicit hand-ordered
    ladder: the bytes that gate the first matmuls (x0, W k0-3) go first,
    the first transpose sits immediately after them (so its barrier drains
    only 2MB), and the remaining W/x1 loads slide in between transposes at
    exactly the rate the k-outer block-0 consumes them.
  - fp8 W halves the W bytes (4MB): the early-DMA window is the binding
    constraint on when block 1 can start.
  - ~22 dummy matmuls on a scratch tile warm the PE HAM clock-gate
    (4/8 -> 8/8) during the otherwise-idle load window; a dummy ACTIVATE
    preloads the ACT function table off the critical path.
  - Blocks >=1 run n-chunk-outer / k-inner: each [128,512] PSUM chain
    retires after 3.4us, relu frees its bank promptly and the ACT/DVE
    post-processing load is smooth.  Block 0 runs k-outer so it can start
    on W k0-3 while the rest of W streams in.
"""

import sys

if "/opt/trn_rl_repo" not in sys.path:
    sys.path.insert(0, "/opt/trn_rl_repo")

import numpy as np

N_CORES = 8
P = 128
TOK_TOTAL = 16384
TOK = TOK_TOTAL // N_CORES  # 2048 tokens per core
D_IN = 2048
D_OUT = 2048
NK = D_IN // P  # 16 contraction tiles
NM = TOK // P  # 16 token blocks per core
NCHUNK = 512  # psum bank free dim (f32)
NN = D_OUT // NCHUNK  # 4
N_WARM = 30  # HAM warm-up matmuls during the load window
# float32 round-to-nearest-even integer trick: adding 1.5*2^23 puts any
# value in [-2^22, 2^22] into [2^23, 2^24) where the f32 ulp is exactly 1,
# so the add rounds RNE to an integer; subtracting recovers round(x).
MAGIC = 12582912.0  # 1.5 * 2**23

_tile_patched = False


def _patch_tile_drain():
    """walrus in this container rejects >2 sem waits on the TileContext exit
    Drain ("Too many sync wait commands").  Split the excess waits onto
    explicit SP wait_ge instructions (same semantics: all waits complete
    before the semaphore free + final barrier)."""
    global _tile_patched
    if _tile_patched:
        return
    import concourse.tile as tile
    from bass_rust import ScopedClock

    def patched(self, tick_clock, wait_clock):
        nc_ = self.nc
        drain_inst = nc_.sync.drain()
        wait_clock.add_sem_waits(
            drain_inst.ins, ScopedClock({None: tick_clock.global_clock})
        )
        waits = list(drain_inst.ins.sync_info.on_wait or [])
        if len(waits) > 1:
            drain_inst.ins.sync_info.on_wait = waits[:1]
            name_to_sem = {}
            for key, h in self.sems.allocated().items():
                name_to_sem[getattr(h, "name", str(key))] = h
            for w in waits[1:]:
                nc_.sync.wait_ge(name_to_sem[w.ant_name], w.wait_value)
        nc_.all_engine_barrier()
        popped = nc_._tile_sem_poison_stack.pop()
        assert popped is self._sem_poison
        nc_.clear_and_free_semaphores(list(self.sems.allocated().values()))
        nc_.all_engine_barrier()

    tile.TileContext._drain_and_barrier = patched
    _tile_patched = True


def _split_excess_waits(nc, max_waits: int = 1):
    """walrus's setupSyncWait caps the number of semaphore waits a single
    instruction can carry.  Tile's scheduler freely attaches more.  Move the
    excess onto wait-only EventSemaphore carrier instructions inserted just
    before the over-subscribed instruction on the same engine (program order
    on one engine => identical semantics)."""
    from concourse import mybir

    n_split = 0
    for fn in nc.m.functions:
        for bb in fn.blocks:
            insts = bb.instructions
            i = 0
            while i < len(insts):
                inst = insts[i]
                si = getattr(inst, "sync_info", None)
                waits = list(si.on_wait) if (si is not None and si.on_wait) else []
                # The ucode DMA-transpose path does not reliably honor
                # instruction-level sem waits -> move ALL of its waits onto
                # engine-level carriers so the sequencer blocks before
                # pushing the transpose.
                limit = 0 if type(inst).__name__ == "InstDmaTransposeAnt" else max_waits
                if len(waits) <= limit:
                    i += 1
                    continue
                keep = waits[-limit:] if limit else []
                extras = waits[: len(waits) - limit]
                pos = i
                for j in range(0, len(extras), max_waits):
                    ev = mybir.InstEventSemaphore(
                        name=f"wsplit_{inst.name}_{j}_{n_split}",
                        engine=inst.engine,
                        ins=[],
                        outs=[],
                        sync_info=mybir.SyncInfo(
                            on_wait=extras[j : j + max_waits], on_update=[]
                        ),
                    )
                    try:
                        nc.register_instruction(ev, overwrite=True)
                    except Exception:
                        pass
                    insts.insert(pos, ev)
                    pos += 1
                inst.sync_info.on_wait = keep
                n_split += 1
                i = pos + 1
    return n_split


def build_program(w_scale: float):
    """Build the per-core Bass program (same program runs SPMD on all 8
    cores; per-core data arrives via the input map)."""
    import concourse.bass as bass
    import concourse.tile as tile
    from concourse import mybir

    f32 = mybir.dt.float32
    bf16 = mybir.dt.bfloat16
    fp8 = mybir.dt.float8e4
    AF = mybir.ActivationFunctionType
    ALU = mybir.AluOpType
    AX = mybir.AxisListType

    _patch_tile_drain()

    ws_f32 = float(np.float32(w_scale))

    nc = bass.Bass("TRN2", target_bir_lowering=False, debug=False)
    xs = nc.dram_tensor("xs", [TOK, D_IN], f32, kind="ExternalInput").ap()
    wq = nc.dram_tensor("wq", [D_IN, D_OUT], fp8, kind="ExternalInput").ap()
    # y ships bf16: square's output rounded to bf16 costs ~0.1% rel-err
    # (budget 2e-2) and halves the store traffic on the congested DMA path.
    y = nc.dram_tensor("y", [TOK, D_OUT], bf16, kind="ExternalOutput").ap()

    with tile.TileContext(nc) as tc:
        with (
            tc.tile_pool(name="wq", bufs=1) as wq_pool,
            tc.tile_pool(name="xin", bufs=4) as x_pool,
            tc.tile_pool(name="xq", bufs=3) as xq_pool,
            tc.tile_pool(name="xqt", bufs=5) as xqt_pool,
            tc.tile_pool(name="scal", bufs=24) as s_pool,
            tc.tile_pool(name="psum", bufs=8, space="PSUM") as psum_pool,
            tc.tile_pool(name="outa", bufs=4) as a_pool,
            tc.tile_pool(name="outb", bufs=4) as b_pool,
            tc.tile_pool(name="consts", bufs=1) as c_pool,
        ):
            # W ships as fp8 (4MB) -- the early-DMA window is the binding
            # startup constraint.  Blocks 0-2 read the fp8 tile directly
            # (+43ns/MM); meanwhile DVE casts each k-tile once to bf16 in
            # the slack of loop iterations 0-2, and blocks 3+ read the bf16
            # copy at the full 216ns/MM rate.  (bf16-ship: +4MB of early
            # DMA, ~25us slower.  GpSimd casts: 7us/k-tile, way too slow.
            # Front-loaded DVE casts: crowd out the x-quant chains.)
            wq8 = wq_pool.tile([P, NK * D_OUT], fp8)
            wqT = wq_pool.tile([P, NK * D_OUT], bf16)
            cmagic = c_pool.tile([P, 1], f32)
            nc.vector.memset(cmagic[:], MAGIC)
            # scratch operands for HAM warm-up matmuls + ACT table preload
            scratch = c_pool.tile([P, NCHUNK], bf16)
            nc.vector.memset(scratch[:], 0.0)
            swarm = c_pool.tile([P, 1], f32)
            # dummy ACTIVATE: pulls the ACT function-table DMA (~1.3us) off
            # the first real quant's critical path.
            nc.scalar.activation(swarm[:], cmagic[:], AF.Identity, bias=0.0, scale=1.0)

            gfs = {}
            xqts = {}
            xfs = {}
            xqs = {}
            Bs = {}

            def _load_x(m, split=1):
                xf = x_pool.tile([P, D_IN], f32, tag="xf", name=f"xf_{m}")
                h = D_IN // split
                for s in range(split):
                    nc.sync.dma_start(
                        xf[:, s * h : (s + 1) * h],
                        xs[m * P : (m + 1) * P, s * h : (s + 1) * h],
                    )
                xfs[m] = xf

            def _load_w(klo, khi):
                # one 3D-AP DMA: wq8[p, k*2048+j] = wq[128k+p, j], k in [klo,khi)
                dst = wq8[:, klo * D_OUT : khi * D_OUT].rearrange(
                    "p (k j) -> p k j", k=khi - klo
                )
                src = wq[klo * P : khi * P, :].rearrange("(k p) j -> p k j", p=P)
                nc.sync.dma_start(dst, src)

            def cast_w(k):
                nc.vector.tensor_copy(
                    wqT[:, k * D_OUT : (k + 1) * D_OUT],
                    wq8[:, k * D_OUT : (k + 1) * D_OUT],
                )

            def emit_x(m, split=1):
                xf = xfs[m]
                h = D_IN // split
                if split == 1:
                    s0 = s_pool.tile([P, 1], f32, tag="s0", name=f"s0_{m}")
                    nc.vector.tensor_reduce(
                        s0[:], xf[:], AX.X, ALU.max, apply_absolute_value=True
                    )
                else:
                    parts = []
                    for s in range(split):
                        sp = s_pool.tile([P, 1], f32, tag="s0", name=f"s0_{m}_{s}")
                        nc.vector.tensor_reduce(
                            sp[:],
                            xf[:, s * h : (s + 1) * h],
                            AX.X,
                            ALU.max,
                            apply_absolute_value=True,
                        )
                        parts.append(sp)
                    s0 = s_pool.tile([P, 1], f32, tag="s0", name=f"s0_{m}_c")
                    nc.vector.tensor_tensor(s0[:], parts[0][:], parts[1][:], ALU.max)
                s1 = s_pool.tile([P, 1], f32, tag="s1", name=f"s1_{m}")
                nc.vector.tensor_scalar(s1[:], s0[:], 1e-5, None, ALU.max)
                rf = s_pool.tile([P, 1], f32, tag="rf", name=f"rf_{m}")
                nc.vector.reciprocal(rf[:], s1[:])
                qf = s_pool.tile([P, 1], f32, tag="qf", name=f"qf_{m}")
                nc.vector.tensor_scalar(qf[:], rf[:], 127.0, None, ALU.mult)
                gf = s_pool.tile([P, 1], f32, tag="gf", name=f"gf_{m}")
                nc.vector.tensor_scalar(gf[:], rf[:], ws_f32, None, ALU.mult)
                gfs[m] = gf
                # x_q = round(x * 127/s): magic add on ACT (in place over xf),
                # magic subtract + bf16 cast on DVE; halves pipelined when the
                # chain is on the startup critical path.
                xq = xq_pool.tile([P, D_IN], bf16, tag="xq", name=f"xq_{m}")
                for s in range(split):
                    sl = slice(s * h, (s + 1) * h)
                    nc.scalar.activation(
                        xf[:, sl],
                        xf[:, sl],
                        AF.Identity,
                        bias=cmagic[:, 0:1],
                        scale=qf[:, 0:1],
                    )
                    nc.vector.tensor_scalar(xq[:, sl], xf[:, sl], MAGIC, None, ALU.subtract)
                xqs[m] = xq

            def emit_transpose(m, split=1):
                # xbar transpose: xqt[p, k, t] = xq[t, 128k+p].  split>1 cuts
                # it into column windows so early k-slices land sooner.
                xqt = xqt_pool.tile([P, D_IN], bf16, tag="xqt", name=f"xqt_{m}")
                kw = NK // split
                for sidx in range(split):
                    lo, hi = sidx * kw * P, (sidx + 1) * kw * P
                    nc.sync.dma_start_transpose(
                        xqt[:, lo:hi].rearrange("p (k t) -> p k t", k=kw),
                        xqs[m][:, lo:hi],
                    )
                xqts[m] = xqt

            def emit_xchain(m):
                _load_x(m, split=2)
                emit_x(m, split=2)
                emit_transpose(m)

            def _postproc(m, n, ps, B, gf, last):
                # out = (ws/s * relu(acc))^2
                A = a_pool.tile([P, NCHUNK], f32, tag="A", name=f"A_{m}_{n}")
                nc.scalar.activation(
                    A[:], ps[:], AF.Relu, bias=0.0, scale=gf[:, 0:1]
                )
                nc.vector.tensor_tensor(
                    B[:, n * NCHUNK : (n + 1) * NCHUNK], A[:], A[:], ALU.mult
                )
                if last:
                    nc.sync.dma_start(
                        y[m * P : (m + 1) * P, n * NCHUNK : (n + 1) * NCHUNK],
                        B[:, n * NCHUNK : (n + 1) * NCHUNK],
                    )

            def compute_block(m, k_outer=False, last=False):
                wsrc = wqT
                xqt = xqts[m]
                gf = gfs[m]
                B = b_pool.tile([P, D_OUT], bf16, tag="B", name=f"B_{m}")
                if k_outer:
                    psums = [
                        psum_pool.tile([P, NCHUNK], f32, tag="ps", name=f"ps_{m}_{n}")
                        for n in range(NN)
                    ]
                    for k in range(NK):
                        for n in range(NN):
                            off = k * D_OUT + n * NCHUNK
                            nc.tensor.matmul(
                                psums[n][:],
                                xqt[:, k * P : (k + 1) * P],
                                wsrc[:, off : off + NCHUNK],
                                start=(k == 0),
                                stop=(k == NK - 1),
                            )
                    for n in range(NN):
                        _postproc(m, n, psums[n], B, gf, last)
                else:
                    for n in range(NN):
                        ps = psum_pool.tile(
                            [P, NCHUNK], f32, tag="ps", name=f"ps_{m}_{n}"
                        )
                        for k in range(NK):
                            off = k * D_OUT + n * NCHUNK
                            nc.tensor.matmul(
                                ps[:],
                                xqt[:, k * P : (k + 1) * P],
                                wsrc[:, off : off + NCHUNK],
                                start=(k == 0),
                                stop=(k == NK - 1),
                            )
                        _postproc(m, n, ps, B, gf, last)
                if not last:
                    Bs[m] = B

            # ---- HAM warm-up: dummy matmuls span the load window so the
            # real stream starts at 2.4GHz.
            ps_warm = psum_pool.tile([P, NCHUNK], f32, tag="ps", name="ps_warm")
            for _ in range(N_WARM):
                nc.tensor.matmul(
                    ps_warm[:], scratch[:, 0:P], scratch[:], start=True, stop=True
                )

            # ---- startup DMA ladder (every DMA rides the sync queue in this
            # exact order; transposes barrier the DMA stream, so each one is
            # placed where its input is ready and only cheap bytes precede it).
            _load_x(0, split=2)
            _load_w(0, 4)
            emit_x(0, split=2)
            emit_transpose(0, split=2)
            for k in range(4):
                cast_w(k)
            _load_w(4, 6)
            cast_w(4)
            cast_w(5)
            _load_w(6, 8)
            cast_w(6)
            cast_w(7)
            _load_x(1, split=2)
            emit_x(1, split=2)
            _load_w(8, 10)
            cast_w(8)
            cast_w(9)
            _load_w(10, 12)
            _load_w(12, 14)
            _load_w(14, 16)
            for k in range(10, 16):
                cast_w(k)
            emit_transpose(1)
            _load_x(2, split=2)
            emit_x(2, split=2)
            emit_transpose(2)
            _load_x(3, split=2)
            emit_x(3, split=2)
            emit_transpose(3)

            for m in range(NM):
                if m + 4 < NM:
                    emit_xchain(m + 4)
                # y(m-1)'s store is emitted HERE, one iteration late: its
                # data finished last iteration, so the launch carries no
                # wait and never blocks the x-load/transpose queue.
                if m >= 1 and (m - 1) in Bs:
                    nc.sync.dma_start(y[(m - 1) * P : m * P, :], Bs.pop(m - 1)[:])
                compute_block(m, k_outer=(m == 0), last=(m == NM - 1))

    _split_excess_waits(nc)
    return nc


def _host_quant_weight(weight: np.ndarray):
    """w_scale = mean(|W|) and the ternary w_q, computed with jax on CPU so
    they are bit-identical to the reference's jnp graph."""
    try:
        import jax
        import jax.numpy as jnp

        cpu = jax.devices("cpu")[0]
        with jax.default_device(cpu):
            w = jnp.asarray(weight, dtype=jnp.float32)
            ws = jnp.mean(jnp.abs(w))
            w_q = jnp.where(w > 0.5 * ws, 1.0, jnp.where(w < -0.5 * ws, -1.0, 0.0))
            return float(ws), np.asarray(w_q, dtype=np.float32)
    except Exception:
        w = weight.astype(np.float32, copy=False)
        ws = np.float32(np.abs(w).astype(np.float64).mean())
        thr = np.float32(0.5) * ws
        w_q = np.where(w > thr, np.float32(1.0), np.where(w < -thr, np.float32(-1.0), np.float32(0.0)))
        return float(ws), w_q


def make_in_maps(x: np.ndarray, weight: np.ndarray, w_q: np.ndarray):
    import ml_dtypes

    x_flat = np.ascontiguousarray(x.reshape(TOK_TOTAL, D_IN).astype(np.float32, copy=False))
    wqt8 = np.ascontiguousarray(w_q.T).astype(ml_dtypes.float8_e4m3)
    return [
        {"xs": x_flat[c * TOK : (c + 1) * TOK, :], "wq": wqt8} for c in range(N_CORES)
    ]


def run_on_hw(x: np.ndarray, weight: np.ndarray, trace: bool = False):
    """Compile + execute on the 8 NeuronCores.  Returns (y_full, results)."""
    from concourse.bass_utils import run_bass_kernel_spmd

    if trace:
        _install_ntff_hook()
    w_scale, w_q = _host_quant_weight(weight)
    nc = build_program(w_scale)
    in_maps = make_in_maps(x, weight, w_q)
    res = run_bass_kernel_spmd(nc, in_maps, list(range(N_CORES)), trace=trace)
    y_full = np.concatenate(
        [np.asarray(res.results[c]["y"]).astype(np.float32) for c in range(N_CORES)],
        axis=0,
    ).reshape(x.shape[0], x.shape[1], D_OUT)
    return y_full.astype(np.float32, copy=False), res


def _install_ntff_hook():
    """The agent image's antenv package lacks axon_hooks, so NTFF profiling
    silently degrades.  Recreate the hook module (ctypes into
    libaxon_pjrt.so) so run_bass_kernel_spmd(trace=True) works."""
    import types, ctypes, contextlib, os

    if "antenv.axon_hooks" in sys.modules:
        return
    so_path = "/opt/axon/libaxon_pjrt.so"
    if not os.path.exists(so_path):
        return
    lib = ctypes.CDLL(so_path)
    if not hasattr(lib, "axon_start_nrt_profile"):
        return
    lib.axon_start_nrt_profile.argtypes = [
        ctypes.POINTER(ctypes.c_int64),
        ctypes.c_size_t,
    ]
    lib.axon_start_nrt_profile.restype = ctypes.c_int64
    lib.axon_stop_nrt_profile.argtypes = [ctypes.c_char_p]
    lib.axon_stop_nrt_profile.restype = ctypes.c_int64

    @contextlib.contextmanager
    def _hook(output_dir, device_ids):
        import jax

        jax.devices()
        if device_ids:
            ids = (ctypes.c_int64 * len(device_ids))(*device_ids)
            rc = lib.axon_start_nrt_profile(ids, len(device_ids))
        else:
            rc = lib.axon_start_nrt_profile(None, 0)
        if rc != 0:
            raise RuntimeError(f"axon_start_nrt_profile rc={rc}")
        try:
            yield
        finally:
            n = lib.axon_stop_nrt_profile(str(output_dir).encode())
            print(f"profile: {n} file(s) written to {output_dir}", file=sys.stderr)

    mod = types.ModuleType("antenv.axon_hooks")
    mod.get_axon_ntff_profile_hook = lambda: _hook
    mod.set_axon_ntff_profile_hook = lambda h: None
    sys.modules["antenv.axon_hooks"] = mod

    # upload_artifacts needs a coo bucket this container doesn't have;
    # degrade to a no-op so trace processing can proceed locally.
    import concourse.bass_utils as bu

    _orig_upload = bu.upload_artifacts

    def _safe_upload(tmpdir):
        try:
            return _orig_upload(tmpdir)
        except Exception as e:
            print(f"upload_artifacts skipped: {e}", file=sys.stderr)
            return tmpdir

    bu.upload_artifacts = _safe_upload


def kernel(x: np.ndarray, weight: np.ndarray) -> np.ndarray:
    y, _ = run_on_hw(x, weight, trace=False)
    return y


# revision 20
# speedup vs baseline: 1.0037x; 1.0037x over previous
"""BitLinear (activation int8-quant + ternary weight) Trainium2 kernel.

Strategy (8 NeuronCores, token-parallel):
  - x [2,8192,2048] -> flat [16384, 2048]; core c gets a contiguous slice of
    2048 tokens (natural [token, feature] layout).
  - weight is TERNARIZED ON HOST exactly as the reference does (jax-CPU
    w_scale = mean|W|, strict f32 compares against +-0.5*w_scale), then
    shipped host-transposed as fp8e4 wqt = w_q.T in {-1,0,+1} ([in, out],
    4MB instead of 16MB f32).  This is the standard BitNet deployment
    contract (ternary weights are a precomputed artifact of the layer).
    The PE consumes the fp8 tiles directly as the moving matmul operand
    against the bf16 x_q stationary operand (verified exact: all values are
    small integers).
  - Activation quantization stays on device and is bit-exact to the
    reference: per-token absmax (DVE), exact int8 round via the magic-number
    trick (ACT magic-add + DVE magic-sub/bf16-cast), per-token scales
    replicated into the output rescale gf = w_scale/s.
  - bf16/fp8 PE matmuls with exact small-integer operands, fp32 PSUM
    accumulation => bit-exact integer GEMM.  out = square(relu(gf*acc)):
    Relu (scale=gf) on ACT, Square on DVE, one batched [128,2048] output
    DMA per token block (per 512-chunk for the last block to cut the tail).

v4 schedule notes (what the v3 trace taught):
  - The kernel is PE-bound: 1024 N=512 matmuls ~= 221us at 2.4GHz; every
    other engine is sized to stay off the PE's critical path.
  - Tile multiplexes ALL DMAs over 8 shared HW completion semaphores and
    every ucode DMA-transpose acts as a *bidirectional barrier* against
    other DMAs in scheduled order.  Therefore every DMA (loads, transposes,
    stores) is emitted on the ONE sync queue in an explicit hand-ordered
    ladder: the bytes that gate the first matmuls (x0, W k0-3) go first,
    the first transpose sits immediately after them (so its barrier drains
    only 2MB), and the remaining W/x1 loads slide in between transposes at
    exactly the rate the k-outer block-0 consumes them.
  - fp8 W halves the W bytes (4MB): the early-DMA window is the binding
    constraint on when block 1 can start.
  - ~22 dummy matmuls on a scratch tile warm the PE HAM clock-gate
    (4/8 -> 8/8) during the otherwise-idle load window; a dummy ACTIVATE
    preloads the ACT function table off the critical path.
  - Blocks >=1 run n-chunk-outer / k-inner: each [128,512] PSUM chain
    retires after 3.4us, relu frees its bank promptly and the ACT/DVE
    post-processing load is smooth.  Block 0 runs k-outer so it can start
    on W k0-3 while the rest of W streams in.
"""

import sys

if "/opt/trn_rl_repo" not in sys.path:
    sys.path.insert(0, "/opt/trn_rl_repo")

import numpy as np

N_CORES = 8
P = 128
TOK_TOTAL = 16384
TOK = TOK_TOTAL // N_CORES  # 2048 tokens per core
D_IN = 2048
D_OUT = 2048
NK = D_IN // P  # 16 contraction tiles
NM = TOK // P  # 16 token blocks per core
NCHUNK = 512  # psum bank free dim (f32)
NN = D_OUT // NCHUNK  # 4
N_WARM = 30  # HAM warm-up matmuls during the load window
# float32 round-to-nearest-even integer trick: adding 1.5*2^23 puts any
# value in [-2^22, 2^22] into [2^23, 2^24) where the f32 ulp is exactly 1,
# so the add rounds RNE to an integer; subtracting recovers round(x).
MAGIC = 12582912.0  # 1.5 * 2**23

_tile_patched = False


def _patch_tile_drain():
    """walrus in this container rejects >2 sem waits on the TileContext exit
    Drain ("Too many sync wait commands").  Split the excess waits onto
    explicit SP wait_ge instructions (same semantics: all waits complete
    before the semaphore free + final barrier)."""
    global _tile_patched
    if _tile_patched:
        return
    import concourse.tile as tile
    from bass_rust import ScopedClock

    def patched(self, tick_clock, wait_clock):
        nc_ = self.nc
        drain_inst = nc_.sync.drain()
        wait_clock.add_sem_waits(
            drain_inst.ins, ScopedClock({None: tick_clock.global_clock})
        )
        waits = list(drain_inst.ins.sync_info.on_wait or [])
        if len(waits) > 1:
            drain_inst.ins.sync_info.on_wait = waits[:1]
            name_to_sem = {}
            for key, h in self.sems.allocated().items():
                name_to_sem[getattr(h, "name", str(key))] = h
            for w in waits[1:]:
                nc_.sync.wait_ge(name_to_sem[w.ant_name], w.wait_value)
        nc_.all_engine_barrier()
        popped = nc_._tile_sem_poison_stack.pop()
        assert popped is self._sem_poison
        nc_.clear_and_free_semaphores(list(self.sems.allocated().values()))
        nc_.all_engine_barrier()

    tile.TileContext._drain_and_barrier = patched
    _tile_patched = True


def _split_excess_waits(nc, max_waits: int = 1):
    """walrus's setupSyncWait caps the number of semaphore waits a single
    instruction can carry.  Tile's scheduler freely attaches more.  Move the
    excess onto wait-only EventSemaphore carrier instructions inserted just
    before the over-subscribed instruction on the same engine (program order
    on one engine => identical semantics)."""
    from concourse import mybir

    n_split = 0
    for fn in nc.m.functions:
        for bb in fn.blocks:
            insts = bb.instructions
            i = 0
            while i < len(insts):
                inst = insts[i]
                si = getattr(inst, "sync_info", None)
                waits = list(si.on_wait) if (si is not None and si.on_wait) else []
                # The ucode DMA-transpose path does not reliably honor
                # instruction-level sem waits -> move ALL of its waits onto
                # engine-level carriers so the sequencer blocks before
                # pushing the transpose.
                limit = 0 if type(inst).__name__ == "InstDmaTransposeAnt" else max_waits
                if len(waits) <= limit:
                    i += 1
                    continue
                keep = waits[-limit:] if limit else []
                extras = waits[: len(waits) - limit]
                pos = i
                for j in range(0, len(extras), max_waits):
                    ev = mybir.InstEventSemaphore(
                        name=f"wsplit_{inst.name}_{j}_{n_split}",
                        engine=inst.engine,
                        ins=[],
                        outs=[],
                        sync_info=mybir.SyncInfo(
                            on_wait=extras[j : j + max_waits], on_update=[]
                        ),
                    )
                    try:
                        nc.register_instruction(ev, overwrite=True)
                    except Exception:
                        pass
                    insts.insert(pos, ev)
                    pos += 1
                inst.sync_info.on_wait = keep
                n_split += 1
                i = pos + 1
    return n_split


def build_program(w_scale: float):
    """Build the per-core Bass program (same program runs SPMD on all 8
    cores; per-core data arrives via the input map)."""
    import concourse.bass as bass
    import concourse.tile as tile
    from concourse import mybir

    f32 = mybir.dt.float32
    bf16 = mybir.dt.bfloat16
    fp8 = mybir.dt.float8e4
    AF = mybir.ActivationFunctionType
    ALU = mybir.AluOpType
    AX = mybir.AxisListType

    _patch_tile_drain()

    ws_f32 = float(np.float32(w_scale))

    nc = bass.Bass("TRN2", target_bir_lowering=False, debug=False)
    xs = nc.dram_tensor("xs", [TOK, D_IN], f32, kind="ExternalInput").ap()
    wq = nc.dram_tensor("wq", [D_IN, D_OUT], fp8, kind="ExternalInput").ap()
    # y ships bf16: square's output rounded to bf16 costs ~0.1% rel-err
    # (budget 2e-2) and halves the store traffic on the congested DMA path.
    y = nc.dram_tensor("y", [TOK, D_OUT], bf16, kind="ExternalOutput").ap()

    with tile.TileContext(nc) as tc:
        with (
            tc.tile_pool(name="wq", bufs=1) as wq_pool,
            tc.tile_pool(name="xin", bufs=4) as x_pool,
            tc.tile_pool(name="xq", bufs=3) as xq_pool,
            tc.tile_pool(name="xqt", bufs=5) as xqt_pool,
            tc.tile_pool(name="scal", bufs=24) as s_pool,
            tc.tile_pool(name="psum", bufs=8, space="PSUM") as psum_pool,
            tc.tile_pool(name="outa", bufs=4) as a_pool,
            tc.tile_pool(name="outb", bufs=4) as b_pool,
            tc.tile_pool(name="consts", bufs=1) as c_pool,
        ):
            # W ships as fp8 (4MB) -- the early-DMA window is the binding
            # startup constraint.  Blocks 0-2 read the fp8 tile directly
            # (+43ns/MM); meanwhile DVE casts each k-tile once to bf16 in
            # the slack of loop iterations 0-2, and blocks 3+ read the bf16
            # copy at the full 216ns/MM rate.  (bf16-ship: +4MB of early
            # DMA, ~25us slower.  GpSimd casts: 7us/k-tile, way too slow.
            # Front-loaded DVE casts: crowd out the x-quant chains.)
            wq8 = wq_pool.tile([P, NK * D_OUT], fp8)
            wqT = wq_pool.tile([P, NK * D_OUT], bf16)
            cmagic = c_pool.tile([P, 1], f32)
            nc.vector.memset(cmagic[:], MAGIC)
            # scratch operands for HAM warm-up matmuls + ACT table preload
            scratch = c_pool.tile([P, NCHUNK], bf16)
            nc.vector.memset(scratch[:], 0.0)
            swarm = c_pool.tile([P, 1], f32)
            # dummy ACTIVATE: pulls the ACT function-table DMA (~1.3us) off
            # the first real quant's critical path.
            nc.scalar.activation(swarm[:], cmagic[:], AF.Identity, bias=0.0, scale=1.0)

            gfs = {}
            xqts = {}
            xfs = {}
            xqs = {}
            Bs = {}

            def _load_x(m, split=1):
                xf = x_pool.tile([P, D_IN], f32, tag="xf", name=f"xf_{m}")
                h = D_IN // split
                for s in range(split):
                    nc.sync.dma_start(
                        xf[:, s * h : (s + 1) * h],
                        xs[m * P : (m + 1) * P, s * h : (s + 1) * h],
                    )
                xfs[m] = xf

            def _load_w(klo, khi):
                # one 3D-AP DMA: wq8[p, k*2048+j] = wq[128k+p, j], k in [klo,khi)
                dst = wq8[:, klo * D_OUT : khi * D_OUT].rearrange(
                    "p (k j) -> p k j", k=khi - klo
                )
                src = wq[klo * P : khi * P, :].rearrange("(k p) j -> p k j", p=P)
                nc.sync.dma_start(dst, src)

            def cast_w(k):
                nc.vector.tensor_copy(
                    wqT[:, k * D_OUT : (k + 1) * D_OUT],
                    wq8[:, k * D_OUT : (k + 1) * D_OUT],
                )

            def emit_x(m, split=1):
                xf = xfs[m]
                h = D_IN // split
                if split == 1:
                    s0 = s_pool.tile([P, 1], f32, tag="s0", name=f"s0_{m}")
                    nc.vector.tensor_reduce(
                        s0[:], xf[:], AX.X, ALU.max, apply_absolute_value=True
                    )
                else:
                    parts = []
                    for s in range(split):
                        sp = s_pool.tile([P, 1], f32, tag="s0", name=f"s0_{m}_{s}")
                        nc.vector.tensor_reduce(
                            sp[:],
                            xf[:, s * h : (s + 1) * h],
                            AX.X,
                            ALU.max,
                            apply_absolute_value=True,
                        )
                        parts.append(sp)
                    s0 = s_pool.tile([P, 1], f32, tag="s0", name=f"s0_{m}_c")
                    nc.vector.tensor_tensor(s0[:], parts[0][:], parts[1][:], ALU.max)
                s1 = s_pool.tile([P, 1], f32, tag="s1", name=f"s1_{m}")
                nc.vector.tensor_scalar(s1[:], s0[:], 1e-5, None, ALU.max)
                rf = s_pool.tile([P, 1], f32, tag="rf", name=f"rf_{m}")
                nc.vector.reciprocal(rf[:], s1[:])
                qf = s_pool.tile([P, 1], f32, tag="qf", name=f"qf_{m}")
                nc.vector.tensor_scalar(qf[:], rf[:], 127.0, None, ALU.mult)
                gf = s_pool.tile([P, 1], f32, tag="gf", name=f"gf_{m}")
                nc.vector.tensor_scalar(gf[:], rf[:], ws_f32, None, ALU.mult)
                gfs[m] = gf
                # x_q = round(x * 127/s): magic add on ACT (in place over xf),
                # magic subtract + bf16 cast on DVE; halves pipelined when the
                # chain is on the startup critical path.
                xq = xq_pool.tile([P, D_IN], bf16, tag="xq", name=f"xq_{m}")
                for s in range(split):
                    sl = slice(s * h, (s + 1) * h)
                    nc.scalar.activation(
                        xf[:, sl],
                        xf[:, sl],
                        AF.Identity,
                        bias=cmagic[:, 0:1],
                        scale=qf[:, 0:1],
                    )
                    nc.vector.tensor_scalar(xq[:, sl], xf[:, sl], MAGIC, None, ALU.subtract)
                xqs[m] = xq

            def emit_transpose(m, split=1):
                # xbar transpose: xqt[p, k, t] = xq[t, 128k+p].  split>1 cuts
                # it into column windows so early k-slices land sooner.
                xqt = xqt_pool.tile([P, D_IN], bf16, tag="xqt", name=f"xqt_{m}")
                kw = NK // split
                for sidx in range(split):
                    lo, hi = sidx * kw * P, (sidx + 1) * kw * P
                    nc.sync.dma_start_transpose(
                        xqt[:, lo:hi].rearrange("p (k t) -> p k t", k=kw),
                        xqs[m][:, lo:hi],
                    )
                xqts[m] = xqt

            def emit_xchain(m):
                _load_x(m, split=2)
                emit_x(m, split=2)
                emit_transpose(m)

            def _postproc(m, n, ps, B, gf, last):
                # out = (ws/s * relu(acc))^2
                A = a_pool.tile([P, NCHUNK], f32, tag="A", name=f"A_{m}_{n}")
                nc.scalar.activation(
                    A[:], ps[:], AF.Relu, bias=0.0, scale=gf[:, 0:1]
                )
                nc.vector.tensor_tensor(
                    B[:, n * NCHUNK : (n + 1) * NCHUNK], A[:], A[:], ALU.mult
                )
                if last:
                    nc.sync.dma_start(
                        y[m * P : (m + 1) * P, n * NCHUNK : (n + 1) * NCHUNK],
                        B[:, n * NCHUNK : (n + 1) * NCHUNK],
                    )

            def compute_block(m, k_outer=False, last=False):
                # block 0 reads the fp8 staging tile directly: +43ns/MM,
                # but no dependency on the DVE cast chain while W streams.
                wsrc = wq8 if m == 0 else wqT
                xqt = xqts[m]
                gf = gfs[m]
                B = b_pool.tile([P, D_OUT], bf16, tag="B", name=f"B_{m}")
                if k_outer:
                    psums = [
                        psum_pool.tile([P, NCHUNK], f32, tag="ps", name=f"ps_{m}_{n}")
                        for n in range(NN)
                    ]
                    for k in range(NK):
                        for n in range(NN):
                            off = k * D_OUT + n * NCHUNK
                            nc.tensor.matmul(
                                psums[n][:],
                                xqt[:, k * P : (k + 1) * P],
                                wsrc[:, off : off + NCHUNK],
                                start=(k == 0),
                                stop=(k == NK - 1),
                            )
                    for n in range(NN):
                        _postproc(m, n, psums[n], B, gf, last)
                else:
                    for n in range(NN):
                        ps = psum_pool.tile(
                            [P, NCHUNK], f32, tag="ps", name=f"ps_{m}_{n}"
                        )
                        for k in range(NK):
                            off = k * D_OUT + n * NCHUNK
                            nc.tensor.matmul(
                                ps[:],
                                xqt[:, k * P : (k + 1) * P],
                                wsrc[:, off : off + NCHUNK],
                                start=(k == 0),
                                stop=(k == NK - 1),
                            )
                        _postproc(m, n, ps, B, gf, last)
                if not last:
                    Bs[m] = B

            # ---- HAM warm-up: dummy matmuls span the load window so the
            # real stream starts at 2.4GHz.
            ps_warm = psum_pool.tile([P, NCHUNK], f32, tag="ps", name="ps_warm")
            for _ in range(N_WARM):
                nc.tensor.matmul(
                    ps_warm[:], scratch[:, 0:P], scratch[:], start=True, stop=True
                )

            # ---- startup DMA ladder (every DMA rides the sync queue in this
            # exact order; transposes barrier the DMA stream, so each one is
            # placed where its input is ready and only cheap bytes precede it).
            _load_x(0, split=2)
            _load_w(0, 4)
            emit_x(0, split=2)
            emit_transpose(0, split=2)
            for k in range(4):
                cast_w(k)
            _load_w(4, 6)
            cast_w(4)
            cast_w(5)
            _load_w(6, 8)
            cast_w(6)
            cast_w(7)
            _load_x(1, split=2)
            emit_x(1, split=2)
            _load_w(8, 10)
            cast_w(8)
            cast_w(9)
            _load_w(10, 12)
            _load_w(12, 14)
            _load_w(14, 16)
            for k in range(10, 16):
                cast_w(k)
            emit_transpose(1)
            _load_x(2, split=2)
            emit_x(2, split=2)
            emit_transpose(2)
            _load_x(3, split=2)
            emit_x(3, split=2)
            emit_transpose(3)

            for m in range(NM):
                if m + 4 < NM:
                    emit_xchain(m + 4)
                # y(m-1)'s store is emitted HERE, one iteration late: its
                # data finished last iteration, so the launch carries no
                # wait and never blocks the x-load/transpose queue.
                if m >= 1 and (m - 1) in Bs:
                    nc.sync.dma_start(y[(m - 1) * P : m * P, :], Bs.pop(m - 1)[:])
                compute_block(m, k_outer=(m == 0), last=(m == NM - 1))

    _split_excess_waits(nc)
    return nc


def _host_quant_weight(weight: np.ndarray):
    """w_scale = mean(|W|) and the ternary w_q, computed with jax on CPU so
    they are bit-identical to the reference's jnp graph."""
    try:
        import jax
        import jax.numpy as jnp

        cpu = jax.devices("cpu")[0]
        with jax.default_device(cpu):
            w = jnp.asarray(weight, dtype=jnp.float32)
            ws = jnp.mean(jnp.abs(w))
            w_q = jnp.where(w > 0.5 * ws, 1.0, jnp.where(w < -0.5 * ws, -1.0, 0.0))
            return float(ws), np.asarray(w_q, dtype=np.float32)
    except Exception:
        w = weight.astype(np.float32, copy=False)
        ws = np.float32(np.abs(w).astype(np.float64).mean())
        thr = np.float32(0.5) * ws
        w_q = np.where(w > thr, np.float32(1.0), np.where(w < -thr, np.float32(-1.0), np.float32(0.0)))
        return float(ws), w_q


def make_in_maps(x: np.ndarray, weight: np.ndarray, w_q: np.ndarray):
    import ml_dtypes

    x_flat = np.ascontiguousarray(x.reshape(TOK_TOTAL, D_IN).astype(np.float32, copy=False))
    wqt8 = np.ascontiguousarray(w_q.T).astype(ml_dtypes.float8_e4m3)
    return [
        {"xs": x_flat[c * TOK : (c + 1) * TOK, :], "wq": wqt8} for c in range(N_CORES)
    ]


def run_on_hw(x: np.ndarray, weight: np.ndarray, trace: bool = False):
    """Compile + execute on the 8 NeuronCores.  Returns (y_full, results)."""
    from concourse.bass_utils import run_bass_kernel_spmd

    if trace:
        _install_ntff_hook()
    w_scale, w_q = _host_quant_weight(weight)
    nc = build_program(w_scale)
    in_maps = make_in_maps(x, weight, w_q)
    res = run_bass_kernel_spmd(nc, in_maps, list(range(N_CORES)), trace=trace)
    y_full = np.concatenate(
        [np.asarray(res.results[c]["y"]).astype(np.float32) for c in range(N_CORES)],
        axis=0,
    ).reshape(x.shape[0], x.shape[1], D_OUT)
    return y_full.astype(np.float32, copy=False), res


def _install_ntff_hook():
    """The agent image's antenv package lacks axon_hooks, so NTFF profiling
    silently degrades.  Recreate the hook module (ctypes into
    libaxon_pjrt.so) so run_bass_kernel_spmd(trace=True) works."""
    import types, ctypes, contextlib, os

    if "antenv.axon_hooks" in sys.modules:
        return
    so_path = "/opt/axon/libaxon_pjrt.so"
    if not os.path.exists(so_path):
        return
    lib = ctypes.CDLL(so_path)
    if not hasattr(lib, "axon_start_nrt_profile"):
        return
    lib.axon_start_nrt_profile.argtypes = [
        ctypes.POINTER(ctypes.c_int64),
        ctypes.c_size_t,
    ]
    lib.axon_start_nrt_profile.restype = ctypes.c_int64
    lib.axon_stop_nrt_profile.argtypes = [ctypes.c_char_p]
    lib.axon_stop_nrt_profile.restype = ctypes.c_int64

    @contextlib.contextmanager
    def _hook(output_dir, device_ids):
        import jax

        jax.devices()
        if device_ids:
            ids = (ctypes.c_int64 * len(device_ids))(*device_ids)
            rc = lib.axon_start_nrt_profile(ids, len(device_ids))
        else:
            rc = lib.axon_start_nrt_profile(None, 0)
        if rc != 0:
            raise RuntimeError(f"axon_start_nrt_profile rc={rc}")
        try:
            yield
        finally:
            n = lib.axon_stop_nrt_profile(str(output_dir).encode())
            print(f"profile: {n} file(s) written to {output_dir}", file=sys.stderr)

    mod = types.ModuleType("antenv.axon_hooks")
    mod.get_axon_ntff_profile_hook = lambda: _hook
    mod.set_axon_ntff_profile_hook = lambda h: None
    sys.modules["antenv.axon_hooks"] = mod

    # upload_artifacts needs a coo bucket this container doesn't have;
    # degrade to a no-op so trace processing can proceed locally.
    import concourse.bass_utils as bu

    _orig_upload = bu.upload_artifacts

    def _safe_upload(tmpdir):
        try:
            return _orig_upload(tmpdir)
        except Exception as e:
            print(f"upload_artifacts skipped: {e}", file=sys.stderr)
            return tmpdir

    bu.upload_artifacts = _safe_upload


def kernel(x: np.ndarray, weight: np.ndarray) -> np.ndarray:
    y, _ = run_on_hw(x, weight, trace=False)
    return y
